# revision 1
# baseline (speedup 1.0000x reference)
"""Trainium2 bit-packing kernel (ConsolidateBits), v3.

See v2 notes; v3 adds: two fully decoupled compute lanes with separate
tile pools (DVE lane / GPSIMD lane), input DMAs spread over SP, Act and
GPSIMD queues by greedy balance, and per-tile (not per-piece) output
DMAs. Compare ops apply the within-byte bit weight; tree levels L1-L3
are packed tensor_tensor adds (DVE 2x_1p mode); L4/L5 (STT, illegal on
GPSIMD) always run on DVE.
"""

import sys

if "/opt/trn_rl_repo" not in sys.path:
    sys.path.insert(0, "/opt/trn_rl_repo")

import numpy as np

import concourse.bass as bass  # noqa: F401
import concourse.bacc as bacc
import concourse.mybir as mybir
from concourse.tile import TileContext
from concourse.alu_op_type import AluOpType as A
from concourse.bass_utils import run_bass_kernel_spmd

P = 128
N_CORES = 8
ROWS, COLS = 4096, 32768
ROWS_PER_CORE = ROWS // N_CORES   # 512
F = 8192
W = F // 32                       # 256 words / partition / tile
NTILES = (ROWS_PER_CORE * COLS) // (P * F)  # 16

SP, ACT, POOL, DVE = 0, 1, 2, 3

# Tuned static schedule (sim makespan 113.2us (simulated-annealing over engine assignment + piece splits); baseline was 220.9us).
# Items: (tile, w0, w1, in_dma_engine, lane, out_dma_engine|None).
# lane 'd': compares+tree on DVE; lane 'p': on GPSIMD. None out = merged
# into the tile's later piece.
TUNED_SCHEDULE = [
    (0, 0, 64, 0, 'd', 2),
    (0, 64, 128, 1, 'd', 1),
    (0, 128, 192, 0, 'd', 2),
    (0, 192, 256, 0, 'd', 2),
    (1, 0, 64, 1, 'p', 0),
    (1, 64, 128, 2, 'p', 2),
    (1, 128, 192, 1, 'p', 1),
    (1, 192, 256, 1, 'p', 1),
    (3, 0, 64, 0, 'p', None),
    (3, 64, 128, 0, 'p', None),
    (3, 128, 256, 1, 'p', 0),
    (2, 0, 128, 0, 'd', None),
    (2, 128, 192, 1, 'd', None),
    (2, 192, 256, 1, 'd', 1),
    (5, 0, 64, 2, 'p', None),
    (5, 64, 128, 0, 'p', None),
    (5, 128, 256, 1, 'p', 1),
    (4, 0, 64, 0, 'p', None),
    (4, 64, 128, 0, 'p', None),
    (4, 128, 256, 1, 'p', 1),
    (6, 0, 128, 0, 'd', None),
    (6, 128, 256, 1, 'd', 1),
    (8, 0, 64, 0, 'd', None),
    (8, 64, 128, 0, 'd', None),
    (8, 128, 256, 1, 'd', 0),
    (7, 0, 128, 0, 'd', None),
    (7, 128, 192, 0, 'd', None),
    (7, 192, 256, 1, 'd', 0),
    (10, 0, 64, 1, 'd', None),
    (10, 64, 128, 0, 'd', None),
    (10, 128, 192, 1, 'd', None),
    (10, 192, 256, 1, 'd', 2),
    (9, 0, 64, 0, 'p', None),
    (9, 64, 128, 1, 'p', None),
    (9, 128, 192, 0, 'p', None),
    (9, 192, 256, 0, 'p', 0),
    (11, 0, 64, 0, 'p', None),
    (11, 64, 128, 1, 'p', None),
    (11, 128, 256, 1, 'p', 0),
    (12, 0, 128, 0, 'd', None),
    (12, 128, 192, 1, 'd', None),
    (12, 192, 256, 0, 'd', 0),
    (14, 0, 128, 0, 'd', None),
    (14, 128, 192, 1, 'd', None),
    (14, 192, 256, 0, 'd', 0),
    (13, 0, 64, 1, 'p', None),
    (13, 64, 128, 1, 'p', None),
    (13, 128, 192, 0, 'p', None),
    (13, 192, 256, 0, 'p', 0),
    (15, 0, 64, 2, 'p', 1),
    (15, 64, 128, 1, 'p', 0),
    (15, 128, 192, 1, 'd', 0),
    (15, 192, 256, 1, 'd', 1),
]


def make_schedule(n_dd_mid=8, pool_dma_budget=11000, mid_halves=True):
    """Items: (t, w0, w1, in_eng, lane, out_eng_or_None).

    lane 'd': cmp+L1-L3 on DVE; lane 'p': on GPSIMD. L4/L5 on DVE.
    out_eng None -> no out DMA for this piece (merged into a later
    piece of the same tile); the last piece of a tile carries the out
    for the whole tile when all pieces share the lane.
    """
    busy = {SP: 0.0, ACT: 0.0, POOL: 0.0}
    def pick(c, pool_ok):
        cands = [SP, ACT] + ([POOL] if pool_ok and busy[POOL] + c <= pool_dma_budget else [])
        e = min(cands, key=lambda k: busy[k])
        busy[e] += c
        return e
    items = []
    # ramp: tile 0 quarters, lanes alternating; per-piece outs
    for q in range(4):
        lane = 'd' if q % 2 == 0 else 'p'
        de = pick(3158, True)
        items.append([0, q * 64, (q + 1) * 64, de, lane, -1])
    # mid tiles 1..14: n_dd_mid DD, rest PP, interleaved
    mids = list(range(1, NTILES - 1))
    n_pp = len(mids) - n_dd_mid
    lanes = []
    dd, pp = n_dd_mid, n_pp
    for i in range(len(mids)):
        if dd * pp == 0:
            lanes.append('d' if dd else 'p'); dd = max(0, dd - 1); pp = max(0, pp - 1)
        elif (i * n_dd_mid) // len(mids) != ((i + 1) * n_dd_mid) // len(mids):
            lanes.append('d'); dd -= 1
        else:
            lanes.append('p'); pp -= 1
    for t, lane in zip(mids, lanes):
        if mid_halves:
            h = W // 2
            de1 = pick(6317, True)
            de2 = pick(6317, True)
            items.append([t, 0, h, de1, lane, None])
            items.append([t, h, W, de2, lane, -1])
        else:
            de = pick(12633, True)
            items.append([t, 0, W, de, lane, -1])
    # drain: tile 15 quarters, alternating lanes, per-piece outs
    for q in range(4):
        lane = 'd' if q % 2 == 0 else 'p'
        de = pick(3158, True)
        items.append([NTILES - 1, q * 64, (q + 1) * 64, de, lane, -1])
    # assign out engines greedily (cost 500 each)
    out = []
    for it in items:
        if it[5] == -1:
            e = pick(500, True)
            it[5] = e
        out.append(tuple(it))
    return out, {k: int(v) for k, v in busy.items()}


def build(schedule=None, xt_bufs=3, bits_bufs=3, small_bufs=3):
    if schedule is None:
        schedule = TUNED_SCHEDULE
    nc = bacc.Bacc("TRN2", target_bir_lowering=False)
    x = nc.dram_tensor("x", [NTILES * P, F], mybir.dt.float32, kind="ExternalInput")
    y = nc.dram_tensor("y", [NTILES * P, W], mybir.dt.int32, kind="ExternalOutput")
    xr = x[:, :].rearrange("(t p) f -> t p f", p=P)
    yr = y[:, :].rearrange("(t p) w -> t p w", p=P)
    f32, bf16, i32 = mybir.dt.float32, mybir.dt.bfloat16, mybir.dt.int32

    with TileContext(nc) as tc:
        with (
            tc.tile_pool(name="consts", bufs=1) as cpool,
            tc.tile_pool(name="xp_d", bufs=xt_bufs) as xp_d,
            tc.tile_pool(name="xp_p", bufs=xt_bufs) as xp_p,
            tc.tile_pool(name="bp_d", bufs=bits_bufs) as bp_d,
            tc.tile_pool(name="bp_p", bufs=bits_bufs) as bp_p,
            tc.tile_pool(name="sp_d", bufs=small_bufs) as sp_d,
            tc.tile_pool(name="sp_p", bufs=small_bufs) as sp_p,
        ):
            shift8 = cpool.tile([P, 1], i32)
            nc.vector.memset(shift8[:], 8)
            engs = [nc.sync, nc.scalar, nc.gpsimd, nc.vector]
            wt_cur = {}   # lane -> (tile, wt buffer, base w0)

            for (t, w0, w1, de, lane, oe) in schedule:
                nw = w1 - w0
                fw = nw * 32
                xpool = xp_d if lane == 'd' else xp_p
                bpool = bp_d if lane == 'd' else bp_p
                spool = sp_d if lane == 'd' else sp_p
                ceng = nc.vector if lane == 'd' else nc.gpsimd

                xt = xpool.tile([P, fw], f32, tag="xt")
                engs[de].dma_start(xt[:], xr[t][:, w0 * 32 : w1 * 32])

                xv = xt[:].rearrange(
                    "p (w k j2 j3 j4) -> p j4 j3 j2 k w",
                    w=nw, k=4, j2=2, j3=2, j4=2,
                )
                bits = bpool.tile([P, fw], bf16, tag="bits")
                for g in range(8):
                    j4, j3, j2 = (g >> 2) & 1, (g >> 1) & 1, g & 1
                    wgt = float(1 << (4 * j2 + 2 * j3 + j4))
                    dst = bits[:, g * 4 * nw : (g + 1) * 4 * nw].rearrange(
                        "p (k w) -> p k w", k=4
                    )
                    if wgt == 1.0:
                        ceng.tensor_scalar(out=dst, in0=xv[:, j4, j3, j2],
                                           scalar1=0.5, scalar2=None, op0=A.is_gt)
                    else:
                        ceng.tensor_scalar(out=dst, in0=xv[:, j4, j3, j2],
                                           scalar1=0.5, scalar2=wgt,
                                           op0=A.is_gt, op1=A.mult)

                u = spool.tile([P, fw // 2], bf16, tag="u")
                ceng.tensor_tensor(out=u[:], in0=bits[:, : fw // 2],
                                   in1=bits[:, fw // 2 :], op=A.add)
                v = spool.tile([P, fw // 4], bf16, tag="v")
                ceng.tensor_tensor(out=v[:], in0=u[:, : fw // 4],
                                   in1=u[:, fw // 4 :], op=A.add)
                xb = spool.tile([P, fw // 8], bf16, tag="xb")
                ceng.tensor_tensor(out=xb[:], in0=v[:, : fw // 8],
                                   in1=v[:, fw // 8 :], op=A.add)
                yh = spool.tile([P, fw // 16], i32, tag="yh")
                nc.vector.scalar_tensor_tensor(
                    out=yh[:], in0=xb[:, fw // 16 :], scalar=65536.0,
                    in1=xb[:, : fw // 16], op0=A.mult, op1=A.add)

                # words: accumulate into a per-tile buffer when pieces of
                # the tile share a lane (oe None on non-final pieces)
                cur = wt_cur.get(lane)
                if cur is None or cur[0] != t:
                    span = W - w0 if oe is None else nw
                    wt = spool.tile([P, span], i32, tag="wt")
                    wt_cur[lane] = (t, wt, w0)
                    base = w0
                else:
                    wt = cur[1]
                    base = cur[2]
                wslot = wt[:, w0 - base : w1 - base]
                nc.vector.scalar_tensor_tensor(
                    out=wslot, in0=yh[:, nw:], scalar=shift8[:],
                    in1=yh[:, :nw], op0=A.logical_shift_left, op1=A.bitwise_or)
                if oe is not None:
                    engs[oe].dma_start(yr[t][:, base:w1], wt[:, : w1 - base])
                    wt_cur[lane] = None

    nc.compile()
    return nc


_NC_CACHE = {}


def _get_nc():
    if "nc" not in _NC_CACHE:
        _NC_CACHE["nc"] = build()
    return _NC_CACHE["nc"]


def _shard(x: np.ndarray):
    return [
        np.ascontiguousarray(
            x[i * ROWS_PER_CORE : (i + 1) * ROWS_PER_CORE].reshape(NTILES * P, F)
        )
        for i in range(N_CORES)
    ]


def run(x: np.ndarray, trace: bool = False):
    nc = _get_nc()
    in_maps = [{"x": s} for s in _shard(x)]
    res = run_bass_kernel_spmd(nc, in_maps, core_ids=list(range(N_CORES)), trace=trace)
    parts = [
        np.asarray(m["y"]).view(np.uint32).reshape(ROWS_PER_CORE, COLS // 32)
        for m in res.results
    ]
    return np.concatenate(parts, axis=0), res


def kernel(x: np.ndarray) -> np.ndarray:
    out, _ = run(np.asarray(x, dtype=np.float32), trace=False)
    return out

